# revision 25
# baseline (speedup 1.0000x reference)
"""Trainium2 Bass kernel for nn_BayesianLayer (dense_mlp).

Reference computation (B=32, R=2, IN=OUT=1024):
    sigma      = softplus(ro)                      # (IN, OUT)
    weights    = eps * sigma + mu                  # (B, R, IN, OUT)
    bias       = eps_b * softplus(ro_b) + mu_b     # (B, R, OUT)
    log_prior  = (mean(ln(mix(weights))) + mean(ln(mix(bias)))) / (B*R)
    log_p      = (mean(ln N(w; mu, sigma)) + mean(ln N(bias; mu_b, sigma_b))) / (B*R)
    out        = (einsum('bi,brio->bro', x, weights) + bias).mean(axis=1)

Device math (per element, w = eps*sigma + mu, q = w^2):
    ln mix(w)      = -q/8 - ln(sqrt(2pi)) + ln(0.5*exp(-3q/8) + 0.25)
    ln N(w;mu,sig) = -eps^2/2 - ln(sigma) - ln(sqrt(2pi))
Each core produces partial sums (Sum q, Sum ln(0.5e+0.25), Sum eps^2,
Sum ln sigma; same four for the bias) plus its 4 rows of `out`; the host
does the final exact float64 combination (the "all-reduce" of the hint).

Sharding: data-parallel over batch. Core c handles batches [4c, 4c+4).

Engine split per half-slab unit (128 x 4096 elements):
    DVE:    s = eps*sigma ; w = s + mu
    ACT:    q = w^2 (in-place over w, +accum) ; e = exp(-3q/8) ;
            ln(0.5e+0.25) (dummy broadcast out, +accum)
    GpSimd: eps^2 (in-place over eps tile, +accum)
    PE:     matvec out += x @ s chunks (fp32r), 2*x @ mu once per batch
"""

import sys

for _p in ("/opt/trn_rl_repo",):
    if _p not in sys.path:
        sys.path.insert(0, _p)

import math
from contextlib import ExitStack

import numpy as np

import concourse.bacc as bacc
import concourse.bass as bass
import concourse.mybir as mybir
import concourse.tile as tile

f32 = mybir.dt.float32
f32r = mybir.dt.float32r
AF = mybir.ActivationFunctionType
OP = mybir.AluOpType

N_CORES = 8
LOG_SQRT_2PI = 0.5 * math.log(2.0 * math.pi)

# Stats layout (partition index of the [16,1] stats output)
S_QW, S_LGW, S_SQW, S_LSW, S_QB, S_LGB, S_SQB, S_LSB = range(8)


def build_kernel(IN=1024, OUT=1024, BPC=4, R=2, HT=4,
                 in_place_q=True, dummy_lg=True, sq_act_frac=0.25,
                 mv_f32r=False, stage=4):
    """Build the SPMD per-core Bass program.

    Per-core DRAM inputs:
      eps      [BPC*R, IT, 128, OUT]    (b,r) slab-major, i = t*128 + p
      xT       [128, IT*BPC]            col t*BPC+b holds x[b, t*128+p]
      mu, ro   [IT, 128, OUT]
      mu_bias, ro_bias [1, OUT]
      eps_bias [128, BRF]               flat (b, o, r) order (host transposes)
    Outputs:
      out      [1, BPC*OUT]
      stats    [16, 1]
    """
    IT = IN // 128
    assert IT % HT == 0
    NH = IT // HT              # half-slabs ("units" chunks) per slab
    OH = max(1, OUT // 512)
    ON = OUT // OH
    U = BPC * R * NH           # total units
    HF = HT * OUT              # free elems per unit tile
    sq_act_elems = int(round(sq_act_frac * HF / 128)) * 128

    RPB = 128 // BPC           # bias layout: rows per batch
    OPR = OUT // RPB           # sigma_b values per row
    BRF = OPR * R              # bias tile free size

    nc = bacc.Bacc("TRN2", target_bir_lowering=False, debug=False,
                   num_devices=N_CORES)

    # activation-bias constant for the mixture ln
    _c = nc.alloc_sbuf_tensor("const-f32-qmix", [128, 1], f32)
    nc.gpsimd.memset(_c.ap(), 0.25)
    nc.const_aps.aps[(f32, 0.25)] = _c.ap()
    nc.all_engine_barrier()

    eps_d = nc.dram_tensor("eps", [BPC * R, IT, 128, OUT], f32, kind="ExternalInput")
    xT_d = nc.dram_tensor("xT", [128, IT * BPC], f32, kind="ExternalInput")
    mu_d = nc.dram_tensor("mu", [IT, 128, OUT], f32, kind="ExternalInput")
    ro_d = nc.dram_tensor("ro", [IT, 128, OUT], f32, kind="ExternalInput")
    mub_d = nc.dram_tensor("mu_bias", [1, OUT], f32, kind="ExternalInput")
    rob_d = nc.dram_tensor("ro_bias", [1, OUT], f32, kind="ExternalInput")
    epsb_d = nc.dram_tensor("eps_bias", [128, BRF], f32, kind="ExternalInput")

    out_d = nc.dram_tensor("out", [1, BPC * OUT], f32, kind="ExternalOutput")
    stats_d = nc.dram_tensor("stats", [16, 1], f32, kind="ExternalOutput")

    with tile.TileContext(nc) as tc, ExitStack() as ctx:
        const = ctx.enter_context(tc.tile_pool(name="const", bufs=1))
        sigmu = ctx.enter_context(tc.tile_pool(name="sigmu", bufs=1))
        epsp = ctx.enter_context(tc.tile_pool(name="epsp", bufs=2))
        sp = ctx.enter_context(tc.tile_pool(name="sp", bufs=2))
        chA = ctx.enter_context(tc.tile_pool(name="chA", bufs=2))
        chB = ctx.enter_context(tc.tile_pool(name="chB", bufs=2))
        psmv = ctx.enter_context(tc.tile_pool(name="psmv", bufs=2, space="PSUM"))
        pssc = ctx.enter_context(tc.tile_pool(name="pssc", bufs=1, space="PSUM"))

        # ---------------- persistent tiles ----------------
        sig = sigmu.tile([128, IT * OUT], f32)
        mu = sigmu.tile([128, IT * OUT], f32)

        accq = const.tile([128, U], f32)
        acclg = const.tile([128, U], f32)
        accsq = const.tile([128, 2 * U], f32)
        accls = const.tile([128, 2], f32)
        for _t in (accq, acclg, accsq, accls):
            nc.vector.memset(_t[:, :], 0.0)
        acc_stack = const.tile([128, 8], f32)
        nc.vector.memset(acc_stack[:, :], 0.0)

        xT = const.tile([128, IT * BPC], f32)
        xT2 = const.tile([128, IT * BPC], f32)
        ones = const.tile([128, 1], f32)
        nc.vector.memset(ones[:, :], 1.0)
        dummy = const.tile([128, 1], f32)

        biash0 = const.tile([1, BPC * OUT], f32)  # bias half-sum, then final out
        stats_sb = const.tile([16, 1], f32)
        if stage < 4:
            nc.vector.memset(biash0[:, :], 0.0)

        nc.sync.dma_start(xT[:, :], xT_d.ap())
        nc.scalar.mul(xT2[:, :], xT[:, :], 2.0)

        # ---------------- mu / sigma setup ----------------
        if stage >= 1:
            nc.sync.dma_start(
                mu[:, :].rearrange("p (t o) -> p t o", t=IT),
                mu_d.ap().rearrange("t p o -> p t o"),
            )
            for m in range(NH):
                sl = slice(m * HF, (m + 1) * HF)
                nc.sync.dma_start(
                    sig[:, sl].rearrange("p (t o) -> p t o", t=HT),
                    ro_d.ap()[m * HT : (m + 1) * HT].rearrange("t p o -> p t o"),
                )
                tmp = chA.tile([128, HF], f32, tag="chA")
                # sigma = softplus(ro) = ln(1 + exp(ro))
                nc.scalar.activation(tmp[:, :], sig[:, sl], AF.Exp)
                nc.scalar.activation(sig[:, sl], tmp[:, :], AF.Ln, bias=1.0, scale=1.0)
            # Sum ln sigma (output discarded via broadcast dummy)
            for m in range(2):
                sl = slice(m * (IT * OUT // 2), (m + 1) * (IT * OUT // 2))
                if dummy_lg:
                    t2o = dummy[:, :].to_broadcast((128, IT * OUT // 2))
                else:
                    t2o = chB.tile([128, IT * OUT // 2], f32, tag="chB", name="lnsig_scr")[:, :]
                nc.scalar.activation(
                    t2o, sig[:, sl], AF.Ln, accum_out=accls[:, m : m + 1]
                )

        # ---------------- bias chain (tiny tiles, all 128 partitions) ------
        # element (b, o, r) lives at row p = b*RPB + o//OPR, col (o%OPR)*R + r
        epsb = const.tile([128, BRF], f32)
        sigb = const.tile([128, OPR], f32)   # no dup; ops broadcast the r dim
        mub = const.tile([128, OPR], f32)
        biasb = const.tile([128, BRF], f32)
        bscr = const.tile([128, BRF], f32)   # scratch (q_b, then reused)
        bscr2 = const.tile([128, BRF], f32)
        biash = const.tile([128, BRF // 2], f32)
        lsb_acc = const.tile([128, 1], f32)
        qb_acc = const.tile([128, 1], f32)
        lgb_acc = const.tile([128, 1], f32)
        sqb_acc = const.tile([128, 1], f32)
        junk = const.tile([128, 1], f32)
        for t_ in (lsb_acc, qb_acc, lgb_acc, sqb_acc):
            nc.vector.memset(t_[:, :], 0.0)
        if stage >= 2:
          nc.sync.dma_start(epsb[:, :], epsb_d.ap())
          for b in range(BPC):
            nc.sync.dma_start(
                sigb[b * RPB : (b + 1) * RPB, :],
                rob_d.ap().rearrange("one (c v) -> (one c) v", c=RPB),
            )
            nc.sync.dma_start(
                mub[b * RPB : (b + 1) * RPB, :],
                mub_d.ap().rearrange("one (c v) -> (one c) v", c=RPB),
            )
          nc.scalar.activation(bscr[:, 0:OPR], sigb[:, :], AF.Exp)
          nc.scalar.activation(sigb[:, :], bscr[:, 0:OPR], AF.Ln, bias=1.0, scale=1.0)
          # Sum ln sigma_b: rows [0, RPB) hold each sigma_b value exactly once
          lnsb_out = const.tile([128, OPR], f32)
          nc.scalar.activation(
            lnsb_out[0:RPB, :], sigb[0:RPB, :], AF.Ln,
            accum_out=lsb_acc[0:RPB, 0:1],
          )
          sigb_b = sigb[:, :].to_broadcast((128, OPR, R))
          mub_b = mub[:, :].to_broadcast((128, OPR, R))
          epsb_3 = epsb[:, :].rearrange("p (v d) -> p v d", d=R)
          biasb_3 = biasb[:, :].rearrange("p (v d) -> p v d", d=R)
          nc.vector.tensor_tensor(biasb_3, epsb_3, sigb_b, OP.mult)
          nc.vector.tensor_tensor(biasb_3, biasb_3, mub_b, OP.add)
          nc.scalar.activation(bscr[:, :], biasb[:, :], AF.Square,
                             accum_out=qb_acc[:, 0:1])
          nc.scalar.activation(bscr2[:, :], bscr[:, :], AF.Exp, scale=-0.375)
          nc.scalar.activation(bscr[:, :], bscr2[:, :], AF.Ln, bias=0.25, scale=0.5,
                             accum_out=lgb_acc[:, 0:1])
          nc.scalar.activation(bscr2[:, :], epsb[:, :], AF.Square,
                             accum_out=sqb_acc[:, 0:1])
          # biash = 0.5*(bias_r0 + bias_r1) ; layout (p, v) = b-major o-order
          assert R == 2
          nc.vector.tensor_tensor(
            biash[:, :], biasb[:, 0 : BRF : 2], biasb[:, 1 : BRF : 2], OP.add
          )
          nc.scalar.mul(biash[:, :], biash[:, :], 0.5)
          nc.sync.dma_start(biash0[:, :], biash[:, :])

        # ---------------- main loop over half-slab units ----------------
        for b in range(BPC if stage >= 3 else 0):
            ps_b = psmv.tile([1, OUT], f32)
            for t in range(IT if stage >= 4 else 0):
                for oh in range(OH):
                    nc.tensor.matmul(
                        ps_b[0:1, oh * ON : (oh + 1) * ON],
                        lhsT=xT2[:, t * BPC + b : t * BPC + b + 1],
                        rhs=mu[:, t * OUT + oh * ON : t * OUT + (oh + 1) * ON],
                        start=(t == 0),
                        stop=False,
                    )
            for r in range(R):
                s8 = b * R + r
                for h in range(NH):
                    u = (b * R + r) * NH + h
                    gt0 = h * HT
                    hsl = slice(gt0 * OUT, (gt0 + HT) * OUT)
                    ep = epsp.tile([128, HF], f32, tag="ep")
                    nc.sync.dma_start(
                        ep[:, :].rearrange("p (t o) -> p t o", t=HT),
                        eps_d.ap()[s8, gt0 : gt0 + HT].rearrange("t p o -> p t o"),
                    )
                    s = sp.tile([128, HF], f32, tag="s")
                    nc.vector.tensor_mul(s[:, :], ep[:, :], sig[:, hsl])
                    w = chA.tile([128, HF], f32, tag="chA")
                    nc.vector.tensor_add(w[:, :], s[:, :], mu[:, hsl])
                    if in_place_q:
                        q = w
                    else:
                        q = chB.tile([128, HF], f32, tag="chB")
                    nc.scalar.activation(
                        q[:, :], w[:, :], AF.Square, accum_out=accq[:, u : u + 1]
                    )
                    e = chA.tile([128, HF], f32, tag="chA")
                    nc.scalar.activation(e[:, :], q[:, :], AF.Exp, scale=-0.375)
                    if dummy_lg:
                        lg_out = dummy[:, :].to_broadcast((128, HF))
                    else:
                        lg_out = chB.tile([128, HF], f32, tag="chB", name="lg_scr")[:, :]
                    nc.scalar.activation(
                        lg_out, e[:, :], AF.Ln, bias=0.25, scale=0.5,
                        accum_out=acclg[:, u : u + 1],
                    )
                    # Sum eps^2: split between ACT (Square+accum on a slice)
                    # and DVE (fused tensor_tensor_reduce, in place) so the
                    # engines stay balanced.
                    c = sq_act_elems
                    if c > 0:
                        if dummy_lg:
                            sq_out = dummy[:, :].to_broadcast((128, c))
                        else:
                            sq_out = chB.tile([128, HF], f32, tag="chB", name="sq_scr")[:, 0:c]
                        nc.scalar.activation(
                            sq_out, ep[:, 0:c], AF.Square,
                            accum_out=accsq[:, 2 * u : 2 * u + 1],
                        )
                    if c < HF:
                        nc.vector.affine_mul_reduce(
                            out=ep[:, c:HF],
                            accum_out=accsq[:, 2 * u + 1 : 2 * u + 2],
                            in0=ep[:, c:HF],
                            in1=ep[:, c:HF],
                            scale=1.0,
                            bias=0.0,
                        )
                    for lt in range(HT if stage >= 4 else 0):
                        gt = gt0 + lt
                        for oh in range(OH):
                            last = (r == R - 1) and (h == NH - 1) and (lt == HT - 1)
                            if mv_f32r:
                                lhs_ap = xT[:, gt * BPC + b : gt * BPC + b + 1].bitcast(f32r)
                                rhs_ap = s[:, lt * OUT + oh * ON : lt * OUT + (oh + 1) * ON].bitcast(f32r)
                            else:
                                lhs_ap = xT[:, gt * BPC + b : gt * BPC + b + 1]
                                rhs_ap = s[:, lt * OUT + oh * ON : lt * OUT + (oh + 1) * ON]
                            nc.tensor.matmul(
                                ps_b[0:1, oh * ON : (oh + 1) * ON],
                                lhsT=lhs_ap,
                                rhs=rhs_ap,
                                start=False,
                                stop=last,
                            )
            # out_b = 0.5*psum + bias_half   (in place over biash0)
            if stage >= 4:
              nc.vector.scalar_tensor_tensor(
                out=biash0[0:1, b * OUT : (b + 1) * OUT],
                in0=ps_b[0:1, :],
                scalar=0.5,
                in1=biash0[0:1, b * OUT : (b + 1) * OUT],
                op0=OP.mult,
                op1=OP.add,
            )

        # ---------------- final reductions ----------------
        nc.vector.tensor_reduce(
            acc_stack[:, S_QW : S_QW + 1], accq[:, :], mybir.AxisListType.X, OP.add
        )
        nc.vector.tensor_reduce(
            acc_stack[:, S_LGW : S_LGW + 1], acclg[:, :], mybir.AxisListType.X, OP.add
        )
        nc.vector.tensor_reduce(
            acc_stack[:, S_SQW : S_SQW + 1], accsq[:, :], mybir.AxisListType.X, OP.add
        )
        nc.vector.tensor_reduce(
            acc_stack[:, S_LSW : S_LSW + 1], accls[:, :], mybir.AxisListType.X, OP.add
        )
        nc.vector.tensor_copy(acc_stack[:, S_QB : S_QB + 1], qb_acc[:, :])
        nc.vector.tensor_copy(acc_stack[:, S_LGB : S_LGB + 1], lgb_acc[:, :])
        nc.vector.tensor_copy(acc_stack[:, S_SQB : S_SQB + 1], sqb_acc[:, :])
        nc.vector.tensor_copy(acc_stack[:, S_LSB : S_LSB + 1], lsb_acc[:, :])

        pss = pssc.tile([8, 1], f32)
        nc.tensor.matmul(
            pss[:, :], lhsT=acc_stack[:, :], rhs=ones[:, :], start=True, stop=True
        )
        nc.vector.memset(stats_sb[:, :], 0.0)
        nc.scalar.copy(stats_sb[0:8, :], pss[:, :])

        nc.sync.dma_start(stats_d.ap(), stats_sb[:, :])
        nc.sync.dma_start(out_d.ap(), biash0[:, :])

    nc.compile()
    return nc


_NC_CACHE = {}


def _get_nc():
    key = "full"
    if key not in _NC_CACHE:
        _NC_CACHE[key] = build_kernel()
    return _NC_CACHE[key]


def make_in_maps(x, mu, ro, mu_bias, ro_bias, eps, eps_bias, n_cores=N_CORES):
    B, Rr, IN, OUT = eps.shape
    BPC = B // n_cores
    IT = IN // 128
    in_maps = []
    for c in range(n_cores):
        bs = slice(c * BPC, (c + 1) * BPC)
        xt = x[bs].T.reshape(IT, 128, BPC).transpose(1, 0, 2).reshape(128, IT * BPC)
        # bias eps: (b, r, o) -> flat (b, o, r) -> (128, BRF)
        eb = eps_bias[bs].transpose(0, 2, 1).reshape(128, -1)
        in_maps.append(
            {
                "eps": np.ascontiguousarray(eps[bs].reshape(BPC * Rr, IT, 128, OUT)),
                "xT": np.ascontiguousarray(xt),
                "mu": np.ascontiguousarray(mu.reshape(IT, 128, OUT)),
                "ro": np.ascontiguousarray(ro.reshape(IT, 128, OUT)),
                "mu_bias": np.ascontiguousarray(mu_bias),
                "ro_bias": np.ascontiguousarray(ro_bias),
                "eps_bias": np.ascontiguousarray(eb),
            }
        )
    return in_maps


def combine_outputs(results, B, Rr, IN, OUT, n_cores=N_CORES):
    BPC = B // n_cores
    out = np.concatenate(
        [r["out"].reshape(BPC, OUT) for r in results], axis=0
    ).astype(np.float32)

    st = np.stack([r["stats"].reshape(16)[:8].astype(np.float64) for r in results])
    n_w_tot = float(B * Rr * IN * OUT)
    n_b_tot = float(B * Rr * OUT)
    denom = float(B * Rr)

    sum_q_w = st[:, S_QW].sum()
    sum_lg_w = st[:, S_LGW].sum()
    sum_sq_w = st[:, S_SQW].sum()
    sum_ls_w = st[0, S_LSW]          # identical on every core
    sum_q_b = st[:, S_QB].sum()
    sum_lg_b = st[:, S_LGB].sum()
    sum_sq_b = st[:, S_SQB].sum()
    sum_ls_b = st[:, S_LSB].mean() * n_cores / n_cores  # per-core identical
    sum_ls_b = st[0, S_LSB]

    mean_lnmix_w = (sum_lg_w - sum_q_w / 8.0) / n_w_tot - LOG_SQRT_2PI
    mean_lnmix_b = (sum_lg_b - sum_q_b / 8.0) / n_b_tot - LOG_SQRT_2PI
    log_prior = (mean_lnmix_w + mean_lnmix_b) / denom

    mean_lpw_w = -sum_sq_w / (2.0 * n_w_tot) - sum_ls_w / (IN * OUT) - LOG_SQRT_2PI
    mean_lpw_b = -sum_sq_b / (2.0 * n_b_tot) - sum_ls_b / OUT - LOG_SQRT_2PI
    log_p_weights = (mean_lpw_w + mean_lpw_b) / denom

    return out, np.float32(log_prior), np.float32(log_p_weights)


def kernel(x, mu, ro, mu_bias, ro_bias, eps, eps_bias, trace=False):
    from concourse.bass_utils import run_bass_kernel_spmd

    x = np.asarray(x, dtype=np.float32)
    mu = np.asarray(mu, dtype=np.float32)
    ro = np.asarray(ro, dtype=np.float32)
    mu_bias = np.asarray(mu_bias, dtype=np.float32)
    ro_bias = np.asarray(ro_bias, dtype=np.float32)
    eps = np.asarray(eps, dtype=np.float32)
    eps_bias = np.asarray(eps_bias, dtype=np.float32)

    B, Rr, IN, OUT = eps.shape
    nc = _get_nc()
    in_maps = make_in_maps(x, mu, ro, mu_bias, ro_bias, eps, eps_bias)
    res = run_bass_kernel_spmd(
        nc, in_maps, core_ids=list(range(N_CORES)), trace=trace
    )
    out, log_prior, log_p = combine_outputs(res.results, B, Rr, IN, OUT)
    kernel.last_results = res
    return out, log_prior, log_p
